# revision 3
# baseline (speedup 1.0000x reference)
"""Trainium2 Bass kernel for batched ResGatedGraphConv.

Reference computation (per (b*t) slice, identical graph across slices):
    k = x @ Wk + bk; q = x @ Wq + bq; v = x @ Wv + bv
    msg_e = leaky_relu(k[dst_e] + q[src_e], 0.01) * v[src_e]
    agg[n] = sum_{e: dst_e == n} msg_e
    out = agg + x @ Wskip + bias

Strategy (8 cores, data-parallel over the 48 (b*t) slices, 6 slices/core):
  - All gathers and the scatter-add are done on the TensorEngine as one-hot
    matmuls. Edges are sorted by (dst_tile, src_tile) 128-node blocks and
    padded per block to multiples of 128 so every 128-edge chunk touches a
    single dst node-tile I and a single src node-tile J.
  - Per chunk: z = oh_dn.T @ k(I) + oh_sn.T @ q(J)   (PSUM accumulate)
               vg = oh_sn.T @ v(J)
               zl = Lrelu(z)          (ScalarE, alpha=0.01)
               msg = zl * vg          (VectorE)
               agg(I) += oh_ed.T @ msg (PSUM accumulate across chunks of I)
  - The 6 slices per core ride in the matmul free dimension (6*64 = 384),
    so every one-hot is reused across all 6 slices.
  - One-hot matrices are precomputed on the host (bf16, exact 0/1) and
    streamed in via DMA. Features use dtype float32r (full-rate PE).
  - Projection biases are folded in via an appended ones-row on x^T.
"""

import numpy as np
import ml_dtypes

B, T, N, F, E = 4, 12, 2048, 64, 32768
NCORES = 8
S = (B * T) // NCORES      # slices per core
NT = N // 128              # node tiles
P = 128
FD = S * F                 # free dim carrying all slices: 384

_prog_cache = {}


def _preprocess_edges(edge_index):
    """Sort edges by (dst_tile, src_tile), pad each block to a multiple of
    128, and build per-chunk one-hot matrices.

    Returns (chunk_flags, ohs):
      chunk_flags: list of (I, J, startI, stopI)
      ohs: [C, 128, 384] bf16; per chunk columns 0:128 = oh_dn [node, edge],
           128:256 = oh_sn [node, edge], 256:384 = oh_ed [edge, node].
    """
    src = np.asarray(edge_index[0], dtype=np.int64)
    dst = np.asarray(edge_index[1], dtype=np.int64)
    ti = dst >> 7
    tj = src >> 7
    key = ti * NT + tj
    order = np.argsort(key, kind="stable")
    s_sorted = (src[order] & 127).astype(np.int64)
    d_sorted = (dst[order] & 127).astype(np.int64)
    k_sorted = key[order]

    uniq, starts = np.unique(k_sorted, return_index=True)
    bounds = list(starts) + [len(k_sorted)]

    chunks = []
    oh_blocks = []
    for gi, kv in enumerate(uniq):
        i_t = int(kv) // NT
        j_t = int(kv) % NT
        lo, hi = bounds[gi], bounds[gi + 1]
        cnt = hi - lo
        nch = (cnt + 127) // 128
        for ci in range(nch):
            a = lo + ci * 128
            b = min(hi, a + 128)
            m = b - a
            sl = s_sorted[a:b]
            dl = d_sorted[a:b]
            e_idx = np.arange(m)
            oh = np.zeros((P, 3 * P), dtype=np.float32)
            oh[dl, e_idx] = 1.0          # oh_dn: [node, edge]
            oh[sl, P + e_idx] = 1.0      # oh_sn: [node, edge]
            oh[e_idx, 2 * P + dl] = 1.0  # oh_ed: [edge, node]; dummy rows stay 0
            oh_blocks.append(oh)
            chunks.append((i_t, j_t))

    C = len(chunks)
    flags = []
    for c, (i_t, j_t) in enumerate(chunks):
        start = c == 0 or chunks[c - 1][0] != i_t
        stop = c == C - 1 or chunks[c + 1][0] != i_t
        flags.append((i_t, j_t, start, stop))
    ohs = np.stack(oh_blocks)  # float32, interpreted as float32r on device
    return flags, ohs


def _build_program(flags):
    import concourse.bass as bass  # noqa: F401
    import concourse.bacc as bacc
    import concourse.mybir as mybir
    import concourse.tile as tile

    f32 = mybir.dt.float32
    f32r = mybir.dt.float32r
    C = len(flags)

    nc = bacc.Bacc(
        "TRN2",
        target_bir_lowering=False,
        debug=False,
        enable_asserts=False,
    )

    xT_d = nc.dram_tensor("xT", [S, F + 1, N], f32r, kind="ExternalInput")
    ohs_d = nc.dram_tensor("ohs", [C, P, 3 * P], f32r, kind="ExternalInput")
    wext_d = nc.dram_tensor("wext", [F + 1, 4 * F], f32r, kind="ExternalInput")
    out_d = nc.dram_tensor("out", [N, FD], f32, kind="ExternalOutput")

    with tile.TileContext(nc) as tc:
        with (
            tc.tile_pool(name="static", bufs=1) as static_pool,
            tc.tile_pool(name="work", bufs=1) as work_pool,
            tc.tile_pool(name="psum", bufs=1, space="PSUM") as psum_pool,
        ):
            # ---- load static data ----
            w_sb = static_pool.tile([F + 1, 4 * F], f32r)
            nc.sync.dma_start(out=w_sb[:], in_=wext_d.ap())
            xs_sb = []
            for s in range(S):
                xt = static_pool.tile([F + 1, N], f32r, name=f"xs{s}")
                nc.sync.dma_start(out=xt[:], in_=xT_d.ap()[s])
                xs_sb.append(xt)

            # ---- projections: proj[n, nt, s, t4, f] ----
            proj_sb = static_pool.tile([P, NT * S * 4 * F], f32r, name="proj")
            proj_ap = proj_sb[:].rearrange(
                "p (nt s t f) -> p nt s t f", nt=NT, s=S, t=4, f=F
            )
            for nt in range(NT):
                for s in range(S):
                    ps = psum_pool.tile([P, 4 * F], f32, tag="proj", bufs=2)
                    nc.tensor.matmul(
                        out=ps[:],
                        lhsT=xs_sb[s][:, nt * P : (nt + 1) * P],
                        rhs=w_sb[:],
                        start=True,
                        stop=True,
                    )
                    dst = proj_ap[:, nt : nt + 1, s : s + 1, :, :]
                    if (nt * S + s) % 2 == 0:
                        nc.scalar.activation(
                            out=dst, in_=ps[:], func=mybir.ActivationFunctionType.Copy
                        )
                    else:
                        nc.vector.tensor_copy(out=dst, in_=ps[:])

            def rhs_ap(i_t, t4):
                return proj_ap[:, i_t : i_t + 1, :, t4 : t4 + 1, :]

            # ---- edge chunks ----
            agg = None
            for c, (i_t, j_t, cstart, cstop) in enumerate(flags):
                oh = work_pool.tile([P, 3 * P], f32r, tag="oh", bufs=12)
                nc.sync.dma_start(out=oh[:], in_=ohs_d.ap()[c])

                z_ps = psum_pool.tile([P, FD], f32, tag="z", bufs=2)
                nc.tensor.matmul(
                    out=z_ps[:],
                    lhsT=oh[:, 0:P],
                    rhs=rhs_ap(i_t, 0),
                    start=True,
                    stop=False,
                )
                nc.tensor.matmul(
                    out=z_ps[:],
                    lhsT=oh[:, P : 2 * P],
                    rhs=rhs_ap(j_t, 1),
                    start=False,
                    stop=True,
                )
                v_ps = psum_pool.tile([P, FD], f32, tag="v", bufs=2)
                nc.tensor.matmul(
                    out=v_ps[:],
                    lhsT=oh[:, P : 2 * P],
                    rhs=rhs_ap(j_t, 2),
                    start=True,
                    stop=True,
                )

                zl = work_pool.tile([P, FD], f32, tag="zl", bufs=3)
                nc.scalar.activation(
                    out=zl[:],
                    in_=z_ps[:],
                    func=mybir.ActivationFunctionType.Lrelu,
                    alpha=0.01,
                )
                msg = work_pool.tile([P, FD], f32r, tag="msg", bufs=3)
                nc.vector.tensor_mul(out=msg[:], in0=zl[:], in1=v_ps[:])

                if cstart:
                    agg = psum_pool.tile([P, FD], f32, tag="agg", bufs=2)
                nc.tensor.matmul(
                    out=agg[:],
                    lhsT=oh[:, 2 * P : 3 * P],
                    rhs=msg[:],
                    start=cstart,
                    stop=cstop,
                )
                if cstop:
                    ot = work_pool.tile([P, FD], f32, tag="ot", bufs=2)
                    nc.vector.tensor_add(
                        out=ot[:], in0=agg[:], in1=rhs_ap(i_t, 3).bitcast(f32)
                    )
                    nc.sync.dma_start(
                        out=out_d.ap()[i_t * P : (i_t + 1) * P, :], in_=ot[:]
                    )

    nc.compile()
    return nc


def kernel(x, edge_index, Wk, bk, Wq, bq, Wv, bv, Wskip, bias):
    from concourse import bass_utils

    x = np.asarray(x, dtype=np.float32)
    edge_index = np.asarray(edge_index)
    xs = x.reshape(B * T, N, F)

    ekey = edge_index.tobytes()
    if ekey not in _prog_cache:
        flags, ohs = _preprocess_edges(edge_index)
        nc = _build_program(flags)
        _prog_cache[ekey] = (nc, ohs)
    nc, ohs = _prog_cache[ekey]

    wext = np.zeros((F + 1, 4 * F), dtype=np.float32)
    for t4, (W, b) in enumerate(
        [(Wk, bk), (Wq, bq), (Wv, bv), (Wskip, bias)]
    ):
        wext[:F, t4 * F : (t4 + 1) * F] = np.asarray(W, dtype=np.float32)
        wext[F, t4 * F : (t4 + 1) * F] = np.asarray(b, dtype=np.float32)

    in_maps = []
    for c in range(NCORES):
        xc = xs[c * S : (c + 1) * S]  # (S, N, F)
        xT = np.ones((S, F + 1, N), dtype=np.float32)
        xT[:, :F, :] = xc.transpose(0, 2, 1)
        in_maps.append({"xT": xT, "ohs": ohs, "wext": wext})

    import os

    trace = os.environ.get("KERNEL_TRACE", "0") == "1"
    res = bass_utils.run_bass_kernel_spmd(
        nc, in_maps, core_ids=list(range(NCORES)), trace=trace
    )
    global last_results
    last_results = res

    outs = []
    for c in range(NCORES):
        o = res.results[c]["out"]  # (N, S*F)
        outs.append(o.reshape(N, S, F).transpose(1, 0, 2))
    full = np.concatenate(outs, axis=0).reshape(B, T, N, F)
    return np.ascontiguousarray(full.astype(np.float32))


# revision 5
# speedup vs baseline: 1.0351x; 1.0351x over previous
"""Trainium2 Bass kernel for batched ResGatedGraphConv.

Reference computation (per (b*t) slice, identical graph across slices):
    k = x @ Wk + bk; q = x @ Wq + bq; v = x @ Wv + bv
    msg_e = leaky_relu(k[dst_e] + q[src_e], 0.01) * v[src_e]
    agg[n] = sum_{e: dst_e == n} msg_e
    out = agg + x @ Wskip + bias

Strategy (8 cores, data-parallel over the 48 (b*t) slices, 6 slices/core):
  - All gathers and the scatter-add run on the TensorEngine as one-hot
    matmuls in float32r (full-rate PE, ~TF32 precision). The 6 slices per
    core ride in the matmul free dimension (6*64 = 384), so every one-hot
    is reused across all 6 slices.
  - Edges are grouped by (dst_tile I, src_tile J) of 128 nodes. Full
    128-edge chunks come from a single (I, J) block; per-I leftovers are
    merged into "tail" chunks that keep a single dst tile I but may span
    several src tiles J — the q/v gathers then accumulate one extra matmul
    per extra J segment.
  - Per chunk: z = oh_dn.T @ k(I) + sum_seg oh_sn.T @ q(J_seg)  (PSUM)
               vg = sum_seg oh_sn.T @ v(J_seg)                  (PSUM)
               zl = Lrelu(z)            (ScalarE, alpha=0.01)
               msg = zl * vg            (VectorE)
               agg(I) += oh_ed.T @ msg  (PSUM accumulate across chunks of I)
  - One-hot 128x128 blocks are precomputed on the host and streamed in via
    one DMA per chunk. Projection biases ride an appended ones-row on x^T.
"""

import numpy as np

B, T, N, F, E = 4, 12, 2048, 64, 32768
NCORES = 8
S = (B * T) // NCORES      # slices per core
NT = N // 128              # node tiles
P = 128
FD = S * F                 # free dim carrying all slices: 384

_prog_cache = {}


def _preprocess_edges(edge_index):
    """Group edges by (dst_tile, src_tile); emit full single-(I,J) chunks
    plus per-I merged tail chunks (single I, multiple J segments).

    Returns (chunks, blocks):
      chunks: list of dicts with keys I, start, stop, blk0 (index of the
        chunk's block run [dn, ed, sn_0, .., sn_{nseg-1}]), segs (list of J).
      blocks: [NB, 128, 128] float32 one-hot blocks, chunk-contiguous.
    """
    src = np.asarray(edge_index[0], dtype=np.int64)
    dst = np.asarray(edge_index[1], dtype=np.int64)
    ti = (dst >> 7).astype(np.int64)
    tj = (src >> 7).astype(np.int64)
    key = ti * NT + tj
    order = np.argsort(key, kind="stable")
    s_l = (src[order] & 127).astype(np.int64)
    d_l = (dst[order] & 127).astype(np.int64)
    k_sorted = key[order]

    uniq, starts = np.unique(k_sorted, return_index=True)
    bounds = np.concatenate([starts, [len(k_sorted)]])
    groups = {int(kv): (int(bounds[gi]), int(bounds[gi + 1]))
              for gi, kv in enumerate(uniq)}

    # raw chunk list: (I, [(J, sl_arr, dl_arr), ...]) — single I per chunk
    raw = []
    for i_t in range(NT):
        # full chunks per (I, J) block + leftover segments
        leftovers = []
        for j_t in range(NT):
            kv = i_t * NT + j_t
            if kv not in groups:
                continue
            lo, hi = groups[kv]
            cnt = hi - lo
            nfull = cnt // 128
            for ci in range(nfull):
                a = lo + ci * 128
                raw.append((i_t, [(j_t, s_l[a:a + 128], d_l[a:a + 128])]))
            rem = cnt - nfull * 128
            if rem:
                a = lo + nfull * 128
                leftovers.append((j_t, s_l[a:hi], d_l[a:hi]))
        # merge leftovers (J-sorted) into chunks of <= 128 edges
        cur, cur_n = [], 0
        for j_t, sl, dl in leftovers:
            pos = 0
            while pos < len(sl):
                take = min(128 - cur_n, len(sl) - pos)
                cur.append((j_t, sl[pos:pos + take], dl[pos:pos + take]))
                cur_n += take
                pos += take
                if cur_n == 128:
                    raw.append((i_t, cur))
                    cur, cur_n = [], 0
        if cur:
            raw.append((i_t, cur))

    blocks = []
    chunks = []
    for i_t, segs in raw:
        dn = np.zeros((P, P), dtype=np.float32)
        ed = np.zeros((P, P), dtype=np.float32)
        sn_blocks = []
        seg_js = []
        e0 = 0
        for j_t, sl, dl in segs:
            m = len(sl)
            e_idx = np.arange(e0, e0 + m)
            dn[dl, e_idx] = 1.0
            ed[e_idx, dl] = 1.0
            sn = np.zeros((P, P), dtype=np.float32)
            sn[sl, e_idx] = 1.0
            sn_blocks.append(sn)
            seg_js.append(j_t)
            e0 += m
        blk0 = len(blocks)
        blocks.append(dn)
        blocks.append(ed)
        blocks.extend(sn_blocks)
        chunks.append({"I": i_t, "blk0": blk0, "segs": seg_js})

    for c, ch in enumerate(chunks):
        ch["start"] = c == 0 or chunks[c - 1]["I"] != ch["I"]
        ch["stop"] = c == len(chunks) - 1 or chunks[c + 1]["I"] != ch["I"]
    return chunks, np.stack(blocks)


def _build_program(chunks, n_blocks, max_nblk):
    import concourse.bacc as bacc
    import concourse.mybir as mybir
    import concourse.tile as tile

    f32 = mybir.dt.float32
    f32r = mybir.dt.float32r

    nc = bacc.Bacc(
        "TRN2",
        target_bir_lowering=False,
        debug=False,
        enable_asserts=False,
    )

    xT_d = nc.dram_tensor("xT", [S, F + 1, N], f32r, kind="ExternalInput")
    ohs_d = nc.dram_tensor("ohs", [P, n_blocks * P], f32r, kind="ExternalInput")
    wext_d = nc.dram_tensor("wext", [F + 1, 4 * F], f32r, kind="ExternalInput")
    out_d = nc.dram_tensor("out", [N, FD], f32, kind="ExternalOutput")

    with tile.TileContext(nc) as tc:
        with (
            tc.tile_pool(name="static", bufs=1) as static_pool,
            tc.tile_pool(name="work", bufs=1) as work_pool,
            tc.tile_pool(name="psum", bufs=1, space="PSUM") as psum_pool,
        ):
            # ---- load static data ----
            w_sb = static_pool.tile([F + 1, 4 * F], f32r)
            nc.sync.dma_start(out=w_sb[:], in_=wext_d.ap())
            xs_sb = []
            for s in range(S):
                xt = static_pool.tile([F + 1, N], f32r, name=f"xs{s}")
                nc.sync.dma_start(out=xt[:], in_=xT_d.ap()[s])
                xs_sb.append(xt)

            # ---- projections: proj[n, nt, s, t4, f] ----
            proj_sb = static_pool.tile([P, NT * S * 4 * F], f32r, name="proj")
            proj_ap = proj_sb[:].rearrange(
                "p (nt s t f) -> p nt s t f", nt=NT, s=S, t=4, f=F
            )
            for nt in range(NT):
                for s in range(S):
                    ps = psum_pool.tile([P, 4 * F], f32, tag="proj", bufs=2)
                    nc.tensor.matmul(
                        out=ps[:],
                        lhsT=xs_sb[s][:, nt * P : (nt + 1) * P],
                        rhs=w_sb[:],
                        start=True,
                        stop=True,
                    )
                    dst = proj_ap[:, nt : nt + 1, s : s + 1, :, :]
                    if (nt * S + s) % 2 == 0:
                        nc.scalar.activation(
                            out=dst, in_=ps[:], func=mybir.ActivationFunctionType.Copy
                        )
                    else:
                        nc.vector.tensor_copy(out=dst, in_=ps[:])

            def rhs_ap(i_t, t4):
                return proj_ap[:, i_t : i_t + 1, :, t4 : t4 + 1, :]

            # ---- edge chunks ----
            ohs_2d = ohs_d.ap()
            agg = None
            for ch in chunks:
                i_t = ch["I"]
                nseg = len(ch["segs"])
                nblk = 2 + nseg
                b0 = ch["blk0"]
                oh = work_pool.tile([P, nblk * P], f32r, tag="oh", bufs=10,
                                    padded_shape=[P, max_nblk * P])
                nc.sync.dma_start(
                    out=oh[:], in_=ohs_2d[:, b0 * P : (b0 + nblk) * P]
                )

                z_ps = psum_pool.tile([P, FD], f32, tag="z", bufs=2)
                nc.tensor.matmul(
                    out=z_ps[:],
                    lhsT=oh[:, 0:P],
                    rhs=rhs_ap(i_t, 0),
                    start=True,
                    stop=False,
                )
                for si, j_t in enumerate(ch["segs"]):
                    nc.tensor.matmul(
                        out=z_ps[:],
                        lhsT=oh[:, (2 + si) * P : (3 + si) * P],
                        rhs=rhs_ap(j_t, 1),
                        start=False,
                        stop=si == nseg - 1,
                    )
                v_ps = psum_pool.tile([P, FD], f32, tag="v", bufs=2)
                for si, j_t in enumerate(ch["segs"]):
                    nc.tensor.matmul(
                        out=v_ps[:],
                        lhsT=oh[:, (2 + si) * P : (3 + si) * P],
                        rhs=rhs_ap(j_t, 2),
                        start=si == 0,
                        stop=si == nseg - 1,
                    )

                zl = work_pool.tile([P, FD], f32, tag="zl", bufs=3)
                nc.scalar.activation(
                    out=zl[:],
                    in_=z_ps[:],
                    func=mybir.ActivationFunctionType.Lrelu,
                    alpha=0.01,
                )
                msg = work_pool.tile([P, FD], f32r, tag="msg", bufs=3)
                nc.vector.tensor_mul(out=msg[:], in0=zl[:], in1=v_ps[:])

                if ch["start"]:
                    agg = psum_pool.tile([P, FD], f32, tag="agg", bufs=2)
                nc.tensor.matmul(
                    out=agg[:],
                    lhsT=oh[:, P : 2 * P],
                    rhs=msg[:],
                    start=ch["start"],
                    stop=ch["stop"],
                )
                if ch["stop"]:
                    ot = work_pool.tile([P, FD], f32, tag="ot", bufs=2)
                    nc.vector.tensor_add(
                        out=ot[:], in0=agg[:], in1=rhs_ap(i_t, 3).bitcast(f32)
                    )
                    nc.sync.dma_start(
                        out=out_d.ap()[i_t * P : (i_t + 1) * P, :], in_=ot[:]
                    )

    nc.compile()
    return nc


def kernel(x, edge_index, Wk, bk, Wq, bq, Wv, bv, Wskip, bias):
    import os

    from concourse import bass_utils

    x = np.asarray(x, dtype=np.float32)
    edge_index = np.asarray(edge_index)
    xs = x.reshape(B * T, N, F)

    ekey = edge_index.tobytes()
    if ekey not in _prog_cache:
        chunks, blocks = _preprocess_edges(edge_index)
        max_nblk = max(2 + len(ch["segs"]) for ch in chunks)
        nc = _build_program(chunks, len(blocks), max_nblk)
        ohs_host = np.ascontiguousarray(
            blocks.transpose(1, 0, 2).reshape(P, -1)
        )
        _prog_cache[ekey] = (nc, ohs_host)
    nc, ohs_host = _prog_cache[ekey]

    wext = np.zeros((F + 1, 4 * F), dtype=np.float32)
    for t4, (W, b) in enumerate(
        [(Wk, bk), (Wq, bq), (Wv, bv), (Wskip, bias)]
    ):
        wext[:F, t4 * F : (t4 + 1) * F] = np.asarray(W, dtype=np.float32)
        wext[F, t4 * F : (t4 + 1) * F] = np.asarray(b, dtype=np.float32)

    in_maps = []
    for c in range(NCORES):
        xc = xs[c * S : (c + 1) * S]  # (S, N, F)
        xT = np.ones((S, F + 1, N), dtype=np.float32)
        xT[:, :F, :] = xc.transpose(0, 2, 1)
        in_maps.append({"xT": xT, "ohs": ohs_host, "wext": wext})

    trace = os.environ.get("KERNEL_TRACE", "0") == "1"
    res = bass_utils.run_bass_kernel_spmd(
        nc, in_maps, core_ids=list(range(NCORES)), trace=trace
    )
    global last_results
    last_results = res

    outs = []
    for c in range(NCORES):
        o = res.results[c]["out"]  # (N, S*F)
        outs.append(o.reshape(N, S, F).transpose(1, 0, 2))
    full = np.concatenate(outs, axis=0).reshape(B, T, N, F)
    return np.ascontiguousarray(full.astype(np.float32))


last_results = None


# revision 6
# speedup vs baseline: 1.1580x; 1.1187x over previous
"""Trainium2 Bass kernel for batched ResGatedGraphConv.

Reference computation (per (b*t) slice, identical graph across slices):
    k = x @ Wk + bk; q = x @ Wq + bq; v = x @ Wv + bv
    msg_e = leaky_relu(k[dst_e] + q[src_e], 0.01) * v[src_e]
    agg[n] = sum_{e: dst_e == n} msg_e
    out = agg + x @ Wskip + bias

Strategy (8 cores, data-parallel over the 48 (b*t) slices, 6 slices/core):
  - All gathers and the scatter-add run on the TensorEngine as one-hot
    matmuls in float32r (full-rate PE, ~TF32 precision). The 6 slices per
    core ride in the matmul free dimension (6*64 = 384), so every one-hot
    is reused across all 6 slices.
  - Edges are grouped by (dst_tile I, src_tile J) of 128 nodes. Full
    128-edge chunks come from a single (I, J) block; per-I leftovers are
    merged into "tail" chunks that keep a single dst tile I but may span
    several src tiles J — the q/v gathers then accumulate one extra matmul
    per extra J segment.
  - Per chunk: z = oh_dn.T @ k(I) + sum_seg oh_sn.T @ q(J_seg)  (PSUM)
               vg = sum_seg oh_sn.T @ v(J_seg)                  (PSUM)
               zl = Lrelu(z)            (ScalarE, alpha=0.01)
               msg = zl * vg            (VectorE)
               agg(I) += oh_ed.T @ msg  (PSUM accumulate across chunks of I)
  - One-hot 128x128 blocks are precomputed on the host and streamed in via
    one DMA per chunk. Projection biases ride an appended ones-row on x^T.
"""

import numpy as np

B, T, N, F, E = 4, 12, 2048, 64, 32768
NCORES = 8
S = (B * T) // NCORES      # slices per core
NT = N // 128              # node tiles
P = 128
FD = S * F                 # free dim carrying all slices: 384

_prog_cache = {}


def _preprocess_edges(edge_index):
    """Group edges by (dst_tile, src_tile); emit full single-(I,J) chunks
    plus per-I merged tail chunks (single I, multiple J segments).

    Returns (chunks, blocks):
      chunks: list of dicts with keys I, start, stop, blk0 (index of the
        chunk's block run [dn, ed, sn_0, .., sn_{nseg-1}]), segs (list of J).
      blocks: [NB, 128, 128] float32 one-hot blocks, chunk-contiguous.
    """
    src = np.asarray(edge_index[0], dtype=np.int64)
    dst = np.asarray(edge_index[1], dtype=np.int64)
    ti = (dst >> 7).astype(np.int64)
    tj = (src >> 7).astype(np.int64)
    key = ti * NT + tj
    order = np.argsort(key, kind="stable")
    s_l = (src[order] & 127).astype(np.int64)
    d_l = (dst[order] & 127).astype(np.int64)
    k_sorted = key[order]

    uniq, starts = np.unique(k_sorted, return_index=True)
    bounds = np.concatenate([starts, [len(k_sorted)]])
    groups = {int(kv): (int(bounds[gi]), int(bounds[gi + 1]))
              for gi, kv in enumerate(uniq)}

    # raw chunk list: (I, [(J, sl_arr, dl_arr), ...]) — single I per chunk
    raw = []
    for i_t in range(NT):
        # full chunks per (I, J) block + leftover segments
        leftovers = []
        for j_t in range(NT):
            kv = i_t * NT + j_t
            if kv not in groups:
                continue
            lo, hi = groups[kv]
            cnt = hi - lo
            nfull = cnt // 128
            for ci in range(nfull):
                a = lo + ci * 128
                raw.append((i_t, [(j_t, s_l[a:a + 128], d_l[a:a + 128])]))
            rem = cnt - nfull * 128
            if rem:
                a = lo + nfull * 128
                leftovers.append((j_t, s_l[a:hi], d_l[a:hi]))
        # first-fit-decreasing pack of leftovers into 128-edge chunks
        # (never split an item: a split doubles its q/v matmul passes)
        bins = []  # (free, [(j, sl, dl), ...])
        for j_t, sl, dl in sorted(leftovers, key=lambda it: -len(it[1])):
            n = len(sl)
            for b in bins:
                if b[0] >= n:
                    b[1].append((j_t, sl, dl))
                    b[0] -= n
                    break
            else:
                bins.append([128 - n, [(j_t, sl, dl)]])
        for _, segs in bins:
            raw.append((i_t, segs))

    blocks = []
    chunks = []
    for i_t, segs in raw:
        dn = np.zeros((P, P), dtype=np.float32)
        ed = np.zeros((P, P), dtype=np.float32)
        sn_blocks = []
        seg_js = []
        e0 = 0
        for j_t, sl, dl in segs:
            m = len(sl)
            e_idx = np.arange(e0, e0 + m)
            dn[dl, e_idx] = 1.0
            ed[e_idx, dl] = 1.0
            sn = np.zeros((P, P), dtype=np.float32)
            sn[sl, e_idx] = 1.0
            sn_blocks.append(sn)
            seg_js.append(j_t)
            e0 += m
        blk0 = len(blocks)
        blocks.append(dn)
        blocks.append(ed)
        blocks.extend(sn_blocks)
        chunks.append({"I": i_t, "blk0": blk0, "segs": seg_js})

    for c, ch in enumerate(chunks):
        ch["start"] = c == 0 or chunks[c - 1]["I"] != ch["I"]
        ch["stop"] = c == len(chunks) - 1 or chunks[c + 1]["I"] != ch["I"]
    return chunks, np.stack(blocks)


def _build_program(chunks, n_blocks, max_nblk):
    import concourse.bacc as bacc
    import concourse.mybir as mybir
    import concourse.tile as tile

    f32 = mybir.dt.float32
    f32r = mybir.dt.float32r

    nc = bacc.Bacc(
        "TRN2",
        target_bir_lowering=False,
        debug=False,
        enable_asserts=False,
    )

    xT_d = nc.dram_tensor("xT", [S, F + 1, N], f32r, kind="ExternalInput")
    ohs_d = nc.dram_tensor("ohs", [P, n_blocks * P], f32r, kind="ExternalInput")
    wext_d = nc.dram_tensor("wext", [F + 1, 4 * F], f32r, kind="ExternalInput")
    out_d = nc.dram_tensor("out", [N, FD], f32, kind="ExternalOutput")

    with tile.TileContext(nc) as tc:
        with (
            tc.tile_pool(name="static", bufs=1) as static_pool,
            tc.tile_pool(name="work", bufs=1) as work_pool,
            tc.tile_pool(name="psum", bufs=1, space="PSUM") as psum_pool,
        ):
            # ---- load static data ----
            w_sb = static_pool.tile([F + 1, 4 * F], f32r)
            nc.sync.dma_start(out=w_sb[:], in_=wext_d.ap())
            xs_sb = []
            for s in range(S):
                xt = static_pool.tile([F + 1, N], f32r, name=f"xs{s}")
                nc.sync.dma_start(out=xt[:], in_=xT_d.ap()[s])
                xs_sb.append(xt)

            # ---- projections: per node-tile proj[nt][p, s, t4, f] ----
            proj_aps = []
            for nt in range(NT):
                pt = static_pool.tile([P, S * 4 * F], f32r, name=f"proj{nt}")
                proj_aps.append(
                    pt[:].rearrange("p (s t f) -> p s t f", s=S, t=4, f=F)
                )
            for nt in range(NT):
                for s in range(S):
                    ps = psum_pool.tile([P, 4 * F], f32, tag="proj", bufs=2)
                    nc.tensor.matmul(
                        out=ps[:],
                        lhsT=xs_sb[s][:, nt * P : (nt + 1) * P],
                        rhs=w_sb[:],
                        start=True,
                        stop=True,
                    )
                    dst = proj_aps[nt][:, s : s + 1, :, :]
                    if (nt * S + s) % 2 == 0:
                        nc.scalar.activation(
                            out=dst, in_=ps[:], func=mybir.ActivationFunctionType.Copy
                        )
                    else:
                        nc.vector.tensor_copy(out=dst, in_=ps[:])

            def rhs_ap(i_t, t4):
                return proj_aps[i_t][:, :, t4 : t4 + 1, :]

            # ---- edge chunks ----
            ohs_2d = ohs_d.ap()
            agg = None
            for ch in chunks:
                i_t = ch["I"]
                nseg = len(ch["segs"])
                nblk = 2 + nseg
                b0 = ch["blk0"]
                oh = work_pool.tile([P, nblk * P], f32r, tag="oh", bufs=10,
                                    padded_shape=[P, max_nblk * P])
                nc.sync.dma_start(
                    out=oh[:], in_=ohs_2d[:, b0 * P : (b0 + nblk) * P]
                )

                z_ps = psum_pool.tile([P, FD], f32, tag="z", bufs=2)
                nc.tensor.matmul(
                    out=z_ps[:],
                    lhsT=oh[:, 0:P],
                    rhs=rhs_ap(i_t, 0),
                    start=True,
                    stop=False,
                )
                for si, j_t in enumerate(ch["segs"]):
                    nc.tensor.matmul(
                        out=z_ps[:],
                        lhsT=oh[:, (2 + si) * P : (3 + si) * P],
                        rhs=rhs_ap(j_t, 1),
                        start=False,
                        stop=si == nseg - 1,
                    )
                v_ps = psum_pool.tile([P, FD], f32, tag="v", bufs=2)
                for si, j_t in enumerate(ch["segs"]):
                    nc.tensor.matmul(
                        out=v_ps[:],
                        lhsT=oh[:, (2 + si) * P : (3 + si) * P],
                        rhs=rhs_ap(j_t, 2),
                        start=si == 0,
                        stop=si == nseg - 1,
                    )

                zl = work_pool.tile([P, FD], f32, tag="zl", bufs=3)
                nc.scalar.activation(
                    out=zl[:],
                    in_=z_ps[:],
                    func=mybir.ActivationFunctionType.Lrelu,
                    alpha=0.01,
                )
                msg = work_pool.tile([P, FD], f32r, tag="msg", bufs=3)
                nc.vector.tensor_mul(out=msg[:], in0=zl[:], in1=v_ps[:])

                if ch["start"]:
                    agg = psum_pool.tile([P, FD], f32, tag="agg", bufs=2)
                nc.tensor.matmul(
                    out=agg[:],
                    lhsT=oh[:, P : 2 * P],
                    rhs=msg[:],
                    start=ch["start"],
                    stop=ch["stop"],
                )
                if ch["stop"]:
                    ot = work_pool.tile([P, FD], f32, tag="ot", bufs=2)
                    nc.vector.tensor_add(
                        out=ot[:], in0=agg[:], in1=rhs_ap(i_t, 3).bitcast(f32)
                    )
                    nc.sync.dma_start(
                        out=out_d.ap()[i_t * P : (i_t + 1) * P, :], in_=ot[:]
                    )

    nc.compile()
    return nc


def kernel(x, edge_index, Wk, bk, Wq, bq, Wv, bv, Wskip, bias):
    import os

    from concourse import bass_utils

    x = np.asarray(x, dtype=np.float32)
    edge_index = np.asarray(edge_index)
    xs = x.reshape(B * T, N, F)

    ekey = edge_index.tobytes()
    if ekey not in _prog_cache:
        chunks, blocks = _preprocess_edges(edge_index)
        max_nblk = max(2 + len(ch["segs"]) for ch in chunks)
        nc = _build_program(chunks, len(blocks), max_nblk)
        ohs_host = np.ascontiguousarray(
            blocks.transpose(1, 0, 2).reshape(P, -1)
        )
        _prog_cache[ekey] = (nc, ohs_host)
    nc, ohs_host = _prog_cache[ekey]

    wext = np.zeros((F + 1, 4 * F), dtype=np.float32)
    for t4, (W, b) in enumerate(
        [(Wk, bk), (Wq, bq), (Wv, bv), (Wskip, bias)]
    ):
        wext[:F, t4 * F : (t4 + 1) * F] = np.asarray(W, dtype=np.float32)
        wext[F, t4 * F : (t4 + 1) * F] = np.asarray(b, dtype=np.float32)

    in_maps = []
    for c in range(NCORES):
        xc = xs[c * S : (c + 1) * S]  # (S, N, F)
        xT = np.ones((S, F + 1, N), dtype=np.float32)
        xT[:, :F, :] = xc.transpose(0, 2, 1)
        in_maps.append({"xT": xT, "ohs": ohs_host, "wext": wext})

    trace = os.environ.get("KERNEL_TRACE", "0") == "1"
    res = bass_utils.run_bass_kernel_spmd(
        nc, in_maps, core_ids=list(range(NCORES)), trace=trace
    )
    global last_results
    last_results = res

    outs = []
    for c in range(NCORES):
        o = res.results[c]["out"]  # (N, S*F)
        outs.append(o.reshape(N, S, F).transpose(1, 0, 2))
    full = np.concatenate(outs, axis=0).reshape(B, T, N, F)
    return np.ascontiguousarray(full.astype(np.float32))


last_results = None


# revision 9
# speedup vs baseline: 1.2281x; 1.0605x over previous
"""Trainium2 Bass kernel for batched ResGatedGraphConv.

Reference computation (per (b*t) slice, identical graph across slices):
    k = x @ Wk + bk; q = x @ Wq + bq; v = x @ Wv + bv
    msg_e = leaky_relu(k[dst_e] + q[src_e], 0.01) * v[src_e]
    agg[n] = sum_{e: dst_e == n} msg_e
    out = agg + x @ Wskip + bias

Strategy (8 cores, data-parallel over the 48 (b*t) slices, 6 slices/core):
  - All gathers and the scatter-add run on the TensorEngine as one-hot
    matmuls in float32r (full-rate PE, ~TF32 precision). The 6 slices per
    core ride in the matmul free dimension (6*64 = 384), so every one-hot
    is reused across all 6 slices.
  - Edges are grouped by (dst_tile I, src_tile J) of 128 nodes. Full
    128-edge chunks come from a single (I, J) block; per-I leftovers are
    merged into "tail" chunks that keep a single dst tile I but may span
    several src tiles J — the q/v gathers then accumulate one extra matmul
    per extra J segment.
  - Per chunk: z = oh_dn.T @ k(I) + sum_seg oh_sn.T @ q(J_seg)  (PSUM)
               vg = sum_seg oh_sn.T @ v(J_seg)                  (PSUM)
               zl = Lrelu(z)            (ScalarE, alpha=0.01)
               msg = zl * vg            (VectorE)
               agg(I) += oh_ed.T @ msg  (PSUM accumulate across chunks of I)
  - One-hot 128x128 blocks are precomputed on the host and streamed in via
    one DMA per chunk. Projection biases ride an appended ones-row on x^T.
"""

import numpy as np

B, T, N, F, E = 4, 12, 2048, 64, 32768
NCORES = 8
S = (B * T) // NCORES      # slices per core
NT = N // 128              # node tiles
P = 128
FD = S * F                 # free dim carrying all slices: 384

_prog_cache = {}


def _preprocess_edges(edge_index):
    """Group edges by (dst_tile, src_tile); emit full single-(I,J) chunks
    plus per-I merged tail chunks (single I, multiple J segments).

    Returns (chunks, blocks):
      chunks: list of dicts with keys I, start, stop, blk0 (index of the
        chunk's block run [dn, ed, sn_0, .., sn_{nseg-1}]), segs (list of J).
      blocks: [NB, 128, 128] float32 one-hot blocks, chunk-contiguous.
    """
    src = np.asarray(edge_index[0], dtype=np.int64)
    dst = np.asarray(edge_index[1], dtype=np.int64)
    ti = (dst >> 7).astype(np.int64)
    tj = (src >> 7).astype(np.int64)
    key = ti * NT + tj
    order = np.argsort(key, kind="stable")
    s_l = (src[order] & 127).astype(np.int64)
    d_l = (dst[order] & 127).astype(np.int64)
    k_sorted = key[order]

    uniq, starts = np.unique(k_sorted, return_index=True)
    bounds = np.concatenate([starts, [len(k_sorted)]])
    groups = {int(kv): (int(bounds[gi]), int(bounds[gi + 1]))
              for gi, kv in enumerate(uniq)}

    # raw chunk list: (I, [(J, sl_arr, dl_arr), ...]) — single I per chunk
    raw = []
    for i_t in range(NT):
        # full chunks per (I, J) block + leftover segments
        leftovers = []
        for j_t in range(NT):
            kv = i_t * NT + j_t
            if kv not in groups:
                continue
            lo, hi = groups[kv]
            cnt = hi - lo
            nfull = cnt // 128
            for ci in range(nfull):
                a = lo + ci * 128
                raw.append((i_t, [(j_t, s_l[a:a + 128], d_l[a:a + 128])]))
            rem = cnt - nfull * 128
            if rem:
                a = lo + nfull * 128
                leftovers.append((j_t, s_l[a:hi], d_l[a:hi]))
        # first-fit-decreasing pack of leftovers into 128-edge chunks
        # (never split an item: a split doubles its q/v matmul passes)
        bins = []  # (free, [(j, sl, dl), ...])
        for j_t, sl, dl in sorted(leftovers, key=lambda it: -len(it[1])):
            n = len(sl)
            for b in bins:
                if b[0] >= n and len(b[1]) < 4:
                    b[1].append((j_t, sl, dl))
                    b[0] -= n
                    break
            else:
                bins.append([128 - n, [(j_t, sl, dl)]])
        for _, segs in bins:
            raw.append((i_t, segs))

    blocks = []
    chunks = []
    for i_t, segs in raw:
        dn = np.zeros((P, P), dtype=np.float32)
        ed = np.zeros((P, P), dtype=np.float32)
        sn_blocks = []
        seg_js = []
        e0 = 0
        for j_t, sl, dl in segs:
            m = len(sl)
            e_idx = np.arange(e0, e0 + m)
            dn[dl, e_idx] = 1.0
            ed[e_idx, dl] = 1.0
            sn = np.zeros((P, P), dtype=np.float32)
            sn[sl, e_idx] = 1.0
            sn_blocks.append(sn)
            seg_js.append(j_t)
            e0 += m
        blk0 = len(blocks)
        blocks.append(dn)
        blocks.append(ed)
        blocks.extend(sn_blocks)
        chunks.append({"I": i_t, "blk0": blk0, "segs": seg_js})

    for c, ch in enumerate(chunks):
        ch["start"] = c == 0 or chunks[c - 1]["I"] != ch["I"]
        ch["stop"] = c == len(chunks) - 1 or chunks[c + 1]["I"] != ch["I"]
    return chunks, np.stack(blocks)


def _build_program(chunks, n_blocks, max_nblk):
    import concourse.bacc as bacc
    import concourse.mybir as mybir
    import concourse.tile as tile

    f32 = mybir.dt.float32
    f32r = mybir.dt.float32r

    nc = bacc.Bacc(
        "TRN2",
        target_bir_lowering=False,
        debug=False,
        enable_asserts=False,
    )

    xT_d = nc.dram_tensor("xT", [S, F + 1, N], f32r, kind="ExternalInput")
    ohs_d = nc.dram_tensor("ohs", [P, n_blocks * P], f32r, kind="ExternalInput")
    wext_d = nc.dram_tensor("wext", [F + 1, 4 * F], f32r, kind="ExternalInput")
    out_d = nc.dram_tensor("out", [N, FD], f32, kind="ExternalOutput")

    with tile.TileContext(nc) as tc:
        with (
            tc.tile_pool(name="static", bufs=1) as static_pool,
            tc.tile_pool(name="work", bufs=1) as work_pool,
            tc.tile_pool(name="psum", bufs=1, space="PSUM") as psum_pool,
        ):
            # ---- load static data ----
            w_sb = static_pool.tile([F + 1, 4 * F], f32r)
            nc.sync.dma_start(out=w_sb[:], in_=wext_d.ap())
            xs_sb = []
            for s in range(S):
                xt = static_pool.tile([F + 1, N], f32r, name=f"xs{s}")
                nc.sync.dma_start(out=xt[:], in_=xT_d.ap()[s])
                xs_sb.append(xt)

            # ---- projections: per node-tile proj[nt][p, s, t4, f] ----
            proj_aps = []
            for nt in range(NT):
                pt = static_pool.tile([P, S * 4 * F], f32r, name=f"proj{nt}")
                proj_aps.append(
                    pt[:].rearrange("p (s t f) -> p s t f", s=S, t=4, f=F)
                )
            for nt in range(NT):
                for s in range(S):
                    ps_full = psum_pool.tile(
                        [P, FD], f32, name="pproj",
                        tag="z" if (nt * S + s) % 2 == 0 else "v",
                        bufs=3,
                    )
                    ps = ps_full[:, : 4 * F]
                    nc.tensor.matmul(
                        out=ps,
                        lhsT=xs_sb[s][:, nt * P : (nt + 1) * P],
                        rhs=w_sb[:],
                        start=True,
                        stop=True,
                    )
                    dst = proj_aps[nt][:, s : s + 1, :, :]
                    if (nt * S + s) % 2 == 0:
                        nc.scalar.activation(
                            out=dst, in_=ps, func=mybir.ActivationFunctionType.Copy
                        )
                    else:
                        nc.vector.tensor_copy(out=dst, in_=ps)

            def rhs_ap(i_t, t4):
                return proj_aps[i_t][:, :, t4 : t4 + 1, :]

            # ---- edge chunks ----
            ohs_2d = ohs_d.ap()
            agg = None
            for ch in chunks:
                i_t = ch["I"]
                nseg = len(ch["segs"])
                nblk = 2 + nseg
                b0 = ch["blk0"]
                oh = work_pool.tile([P, nblk * P], f32r, tag="oh", bufs=8,
                                    padded_shape=[P, max_nblk * P])
                nc.sync.dma_start(
                    out=oh[:], in_=ohs_2d[:, b0 * P : (b0 + nblk) * P]
                )

                z_ps = psum_pool.tile([P, FD], f32, tag="z", bufs=3)
                nc.tensor.matmul(
                    out=z_ps[:],
                    lhsT=oh[:, 0:P],
                    rhs=rhs_ap(i_t, 0),
                    start=True,
                    stop=False,
                )
                for si, j_t in enumerate(ch["segs"]):
                    nc.tensor.matmul(
                        out=z_ps[:],
                        lhsT=oh[:, (2 + si) * P : (3 + si) * P],
                        rhs=rhs_ap(j_t, 1),
                        start=False,
                        stop=si == nseg - 1,
                    )
                v_ps = psum_pool.tile([P, FD], f32, tag="v", bufs=3)
                for si, j_t in enumerate(ch["segs"]):
                    nc.tensor.matmul(
                        out=v_ps[:],
                        lhsT=oh[:, (2 + si) * P : (3 + si) * P],
                        rhs=rhs_ap(j_t, 2),
                        start=si == 0,
                        stop=si == nseg - 1,
                    )

                zl = work_pool.tile([P, FD], f32, tag="zl", bufs=4)
                nc.scalar.activation(
                    out=zl[:],
                    in_=z_ps[:],
                    func=mybir.ActivationFunctionType.Lrelu,
                    alpha=0.01,
                )
                msg = work_pool.tile([P, FD], f32r, tag="msg", bufs=4)
                nc.vector.tensor_mul(out=msg[:], in0=zl[:], in1=v_ps[:])

                if ch["start"]:
                    agg = psum_pool.tile([P, FD], f32, tag="agg", bufs=2)
                nc.tensor.matmul(
                    out=agg[:],
                    lhsT=oh[:, P : 2 * P],
                    rhs=msg[:],
                    start=ch["start"],
                    stop=ch["stop"],
                )
                if ch["stop"]:
                    ot = work_pool.tile([P, FD], f32, tag="ot", bufs=2)
                    nc.vector.tensor_add(
                        out=ot[:], in0=agg[:], in1=rhs_ap(i_t, 3).bitcast(f32)
                    )
                    nc.sync.dma_start(
                        out=out_d.ap()[i_t * P : (i_t + 1) * P, :], in_=ot[:]
                    )

    nc.compile()
    return nc


def kernel(x, edge_index, Wk, bk, Wq, bq, Wv, bv, Wskip, bias):
    import os

    from concourse import bass_utils

    x = np.asarray(x, dtype=np.float32)
    edge_index = np.asarray(edge_index)
    xs = x.reshape(B * T, N, F)

    ekey = edge_index.tobytes()
    if ekey not in _prog_cache:
        chunks, blocks = _preprocess_edges(edge_index)
        max_nblk = max(2 + len(ch["segs"]) for ch in chunks)
        nc = _build_program(chunks, len(blocks), max_nblk)
        ohs_host = np.ascontiguousarray(
            blocks.transpose(1, 0, 2).reshape(P, -1)
        )
        _prog_cache[ekey] = (nc, ohs_host)
    nc, ohs_host = _prog_cache[ekey]

    wext = np.zeros((F + 1, 4 * F), dtype=np.float32)
    for t4, (W, b) in enumerate(
        [(Wk, bk), (Wq, bq), (Wv, bv), (Wskip, bias)]
    ):
        wext[:F, t4 * F : (t4 + 1) * F] = np.asarray(W, dtype=np.float32)
        wext[F, t4 * F : (t4 + 1) * F] = np.asarray(b, dtype=np.float32)

    in_maps = []
    for c in range(NCORES):
        xc = xs[c * S : (c + 1) * S]  # (S, N, F)
        xT = np.ones((S, F + 1, N), dtype=np.float32)
        xT[:, :F, :] = xc.transpose(0, 2, 1)
        in_maps.append({"xT": xT, "ohs": ohs_host, "wext": wext})

    trace = os.environ.get("KERNEL_TRACE", "0") == "1"
    res = bass_utils.run_bass_kernel_spmd(
        nc, in_maps, core_ids=list(range(NCORES)), trace=trace
    )
    global last_results
    last_results = res

    outs = []
    for c in range(NCORES):
        o = res.results[c]["out"]  # (N, S*F)
        outs.append(o.reshape(N, S, F).transpose(1, 0, 2))
    full = np.concatenate(outs, axis=0).reshape(B, T, N, F)
    return np.ascontiguousarray(full.astype(np.float32))


last_results = None


# revision 10
# speedup vs baseline: 1.2484x; 1.0166x over previous
"""Trainium2 Bass kernel for batched ResGatedGraphConv.

Reference computation (per (b*t) slice, identical graph across slices):
    k = x @ Wk + bk; q = x @ Wq + bq; v = x @ Wv + bv
    msg_e = leaky_relu(k[dst_e] + q[src_e], 0.01) * v[src_e]
    agg[n] = sum_{e: dst_e == n} msg_e
    out = agg + x @ Wskip + bias

Strategy (8 cores, data-parallel over the 48 (b*t) slices, 6 slices/core):
  - All gathers and the scatter-add run on the TensorEngine as one-hot
    matmuls in float32r (full-rate PE, ~TF32 precision). The 6 slices per
    core ride in the matmul free dimension (6*64 = 384), so every one-hot
    is reused across all 6 slices.
  - Edges are grouped by (dst_tile I, src_tile J) of 128 nodes. Full
    128-edge chunks come from a single (I, J) block; per-I leftovers are
    merged into "tail" chunks that keep a single dst tile I but may span
    several src tiles J — the q/v gathers then accumulate one extra matmul
    per extra J segment.
  - Per chunk: z = oh_dn.T @ k(I) + sum_seg oh_sn.T @ q(J_seg)  (PSUM)
               vg = sum_seg oh_sn.T @ v(J_seg)                  (PSUM)
               zl = Lrelu(z)            (ScalarE, alpha=0.01)
               msg = zl * vg            (VectorE)
               agg(I) += oh_ed.T @ msg  (PSUM accumulate across chunks of I)
  - One-hot 128x128 blocks are precomputed on the host and streamed in via
    one DMA per chunk. Projection biases ride an appended ones-row on x^T.
"""

import numpy as np

B, T, N, F, E = 4, 12, 2048, 64, 32768
NCORES = 8
S = (B * T) // NCORES      # slices per core
NT = N // 128              # node tiles
P = 128
FD = S * F                 # free dim carrying all slices: 384

_prog_cache = {}


def _preprocess_edges(edge_index):
    """Group edges by (dst_tile, src_tile); emit full single-(I,J) chunks
    plus per-I merged tail chunks (single I, multiple J segments).

    Returns (chunks, blocks):
      chunks: list of dicts with keys I, start, stop, blk0 (index of the
        chunk's block run [dn, ed, sn_0, .., sn_{nseg-1}]), segs (list of J).
      blocks: [NB, 128, 128] float32 one-hot blocks, chunk-contiguous.
    """
    src = np.asarray(edge_index[0], dtype=np.int64)
    dst = np.asarray(edge_index[1], dtype=np.int64)
    ti = (dst >> 7).astype(np.int64)
    tj = (src >> 7).astype(np.int64)
    key = ti * NT + tj
    order = np.argsort(key, kind="stable")
    s_l = (src[order] & 127).astype(np.int64)
    d_l = (dst[order] & 127).astype(np.int64)
    k_sorted = key[order]

    uniq, starts = np.unique(k_sorted, return_index=True)
    bounds = np.concatenate([starts, [len(k_sorted)]])
    groups = {int(kv): (int(bounds[gi]), int(bounds[gi + 1]))
              for gi, kv in enumerate(uniq)}

    # raw chunk list: (I, [(J, sl_arr, dl_arr), ...]) — single I per chunk
    raw = []
    for i_t in range(NT):
        # full chunks per (I, J) block + leftover segments
        leftovers = []
        for j_t in range(NT):
            kv = i_t * NT + j_t
            if kv not in groups:
                continue
            lo, hi = groups[kv]
            cnt = hi - lo
            nfull = cnt // 128
            for ci in range(nfull):
                a = lo + ci * 128
                raw.append((i_t, [(j_t, s_l[a:a + 128], d_l[a:a + 128])]))
            rem = cnt - nfull * 128
            if rem:
                a = lo + nfull * 128
                leftovers.append((j_t, s_l[a:hi], d_l[a:hi]))
        # first-fit-decreasing pack of leftovers into 128-edge chunks
        # (never split an item: a split doubles its q/v matmul passes)
        bins = []  # (free, [(j, sl, dl), ...])
        for j_t, sl, dl in sorted(leftovers, key=lambda it: -len(it[1])):
            n = len(sl)
            for b in bins:
                if b[0] >= n and len(b[1]) < 4:
                    b[1].append((j_t, sl, dl))
                    b[0] -= n
                    break
            else:
                bins.append([128 - n, [(j_t, sl, dl)]])
        for _, segs in bins:
            raw.append((i_t, segs))

    blocks = []
    chunks = []
    for i_t, segs in raw:
        dn = np.zeros((P, P), dtype=np.float32)
        ed = np.zeros((P, P), dtype=np.float32)
        sn_blocks = []
        seg_js = []
        e0 = 0
        for j_t, sl, dl in segs:
            m = len(sl)
            e_idx = np.arange(e0, e0 + m)
            dn[dl, e_idx] = 1.0
            ed[e_idx, dl] = 1.0
            sn = np.zeros((P, P), dtype=np.float32)
            sn[sl, e_idx] = 1.0
            sn_blocks.append(sn)
            seg_js.append(j_t)
            e0 += m
        blk0 = len(blocks)
        blocks.append(dn)
        blocks.append(ed)
        blocks.extend(sn_blocks)
        chunks.append({"I": i_t, "blk0": blk0, "segs": seg_js})

    for c, ch in enumerate(chunks):
        ch["start"] = c == 0 or chunks[c - 1]["I"] != ch["I"]
        ch["stop"] = c == len(chunks) - 1 or chunks[c + 1]["I"] != ch["I"]
    return chunks, np.stack(blocks)


def _build_program(chunks, n_blocks, max_nblk):
    import concourse.bacc as bacc
    import concourse.mybir as mybir
    import concourse.tile as tile

    f32 = mybir.dt.float32
    f32r = mybir.dt.float32r

    nc = bacc.Bacc(
        "TRN2",
        target_bir_lowering=False,
        debug=False,
        enable_asserts=False,
    )

    xT_d = nc.dram_tensor("xT", [S, F + 1, N], f32r, kind="ExternalInput")
    ohs_d = nc.dram_tensor("ohs", [P, n_blocks * P], f32r, kind="ExternalInput")
    wext_d = nc.dram_tensor("wext", [F + 1, 4 * F], f32r, kind="ExternalInput")
    out_d = nc.dram_tensor("out", [N, FD], f32, kind="ExternalOutput")

    with tile.TileContext(nc) as tc:
        with (
            tc.tile_pool(name="static", bufs=1) as static_pool,
            tc.tile_pool(name="psum", bufs=1, space="PSUM") as psum_pool,
        ):
            # ---- load static data ----
            w_sb = static_pool.tile([F + 1, 4 * F], f32r)
            nc.sync.dma_start(out=w_sb[:], in_=wext_d.ap())
            xsp = tc.alloc_tile_pool(name="xsp", bufs=1)
            xs_sb = []
            for s in range(S):
                xt = xsp.tile([F + 1, N], f32r, name=f"xs{s}")
                nc.sync.dma_start(out=xt[:], in_=xT_d.ap()[s])
                xs_sb.append(xt)

            # ---- projections: per node-tile proj[nt][p, s, t4, f] ----
            proj_aps = []
            for nt in range(NT):
                pt = static_pool.tile([P, S * 4 * F], f32r, name=f"proj{nt}")
                proj_aps.append(
                    pt[:].rearrange("p (s t f) -> p s t f", s=S, t=4, f=F)
                )
            for nt in range(NT):
                for s in range(S):
                    ps_full = psum_pool.tile(
                        [P, FD], f32, name="pproj",
                        tag="z" if (nt * S + s) % 2 == 0 else "v",
                        bufs=3,
                    )
                    ps = ps_full[:, : 4 * F]
                    nc.tensor.matmul(
                        out=ps,
                        lhsT=xs_sb[s][:, nt * P : (nt + 1) * P],
                        rhs=w_sb[:],
                        start=True,
                        stop=True,
                    )
                    dst = proj_aps[nt][:, s : s + 1, :, :]
                    if (nt * S + s) % 2 == 0:
                        nc.scalar.activation(
                            out=dst, in_=ps, func=mybir.ActivationFunctionType.Copy
                        )
                    else:
                        nc.vector.tensor_copy(out=dst, in_=ps)

            def rhs_ap(i_t, t4):
                return proj_aps[i_t][:, :, t4 : t4 + 1, :]

            xsp.release()

            # ---- edge chunks ----
            work_pool = tc.alloc_tile_pool(name="work", bufs=1)
            ohs_2d = ohs_d.ap()
            agg = None
            for ch in chunks:
                i_t = ch["I"]
                nseg = len(ch["segs"])
                nblk = 2 + nseg
                b0 = ch["blk0"]
                oh = work_pool.tile([P, nblk * P], f32r, tag="oh", bufs=12,
                                    padded_shape=[P, max_nblk * P])
                nc.sync.dma_start(
                    out=oh[:], in_=ohs_2d[:, b0 * P : (b0 + nblk) * P]
                )

                z_ps = psum_pool.tile([P, FD], f32, tag="z", bufs=3)
                nc.tensor.matmul(
                    out=z_ps[:],
                    lhsT=oh[:, 0:P],
                    rhs=rhs_ap(i_t, 0),
                    start=True,
                    stop=False,
                )
                for si, j_t in enumerate(ch["segs"]):
                    nc.tensor.matmul(
                        out=z_ps[:],
                        lhsT=oh[:, (2 + si) * P : (3 + si) * P],
                        rhs=rhs_ap(j_t, 1),
                        start=False,
                        stop=si == nseg - 1,
                    )
                v_ps = psum_pool.tile([P, FD], f32, tag="v", bufs=3)
                for si, j_t in enumerate(ch["segs"]):
                    nc.tensor.matmul(
                        out=v_ps[:],
                        lhsT=oh[:, (2 + si) * P : (3 + si) * P],
                        rhs=rhs_ap(j_t, 2),
                        start=si == 0,
                        stop=si == nseg - 1,
                    )

                zl = work_pool.tile([P, FD], f32, tag="zl", bufs=6)
                nc.scalar.activation(
                    out=zl[:],
                    in_=z_ps[:],
                    func=mybir.ActivationFunctionType.Lrelu,
                    alpha=0.01,
                )
                msg = work_pool.tile([P, FD], f32r, tag="msg", bufs=6)
                nc.vector.tensor_mul(out=msg[:], in0=zl[:], in1=v_ps[:])

                if ch["start"]:
                    agg = psum_pool.tile([P, FD], f32, tag="agg", bufs=2)
                nc.tensor.matmul(
                    out=agg[:],
                    lhsT=oh[:, P : 2 * P],
                    rhs=msg[:],
                    start=ch["start"],
                    stop=ch["stop"],
                )
                if ch["stop"]:
                    ot = work_pool.tile([P, FD], f32, tag="ot", bufs=2)
                    nc.vector.tensor_add(
                        out=ot[:], in0=agg[:], in1=rhs_ap(i_t, 3).bitcast(f32)
                    )
                    nc.sync.dma_start(
                        out=out_d.ap()[i_t * P : (i_t + 1) * P, :], in_=ot[:]
                    )
            work_pool.release()

    nc.compile()
    return nc


def kernel(x, edge_index, Wk, bk, Wq, bq, Wv, bv, Wskip, bias):
    import os

    from concourse import bass_utils

    x = np.asarray(x, dtype=np.float32)
    edge_index = np.asarray(edge_index)
    xs = x.reshape(B * T, N, F)

    ekey = edge_index.tobytes()
    if ekey not in _prog_cache:
        chunks, blocks = _preprocess_edges(edge_index)
        max_nblk = max(2 + len(ch["segs"]) for ch in chunks)
        nc = _build_program(chunks, len(blocks), max_nblk)
        ohs_host = np.ascontiguousarray(
            blocks.transpose(1, 0, 2).reshape(P, -1)
        )
        _prog_cache[ekey] = (nc, ohs_host)
    nc, ohs_host = _prog_cache[ekey]

    wext = np.zeros((F + 1, 4 * F), dtype=np.float32)
    for t4, (W, b) in enumerate(
        [(Wk, bk), (Wq, bq), (Wv, bv), (Wskip, bias)]
    ):
        wext[:F, t4 * F : (t4 + 1) * F] = np.asarray(W, dtype=np.float32)
        wext[F, t4 * F : (t4 + 1) * F] = np.asarray(b, dtype=np.float32)

    in_maps = []
    for c in range(NCORES):
        xc = xs[c * S : (c + 1) * S]  # (S, N, F)
        xT = np.ones((S, F + 1, N), dtype=np.float32)
        xT[:, :F, :] = xc.transpose(0, 2, 1)
        in_maps.append({"xT": xT, "ohs": ohs_host, "wext": wext})

    trace = os.environ.get("KERNEL_TRACE", "0") == "1"
    res = bass_utils.run_bass_kernel_spmd(
        nc, in_maps, core_ids=list(range(NCORES)), trace=trace
    )
    global last_results
    last_results = res

    outs = []
    for c in range(NCORES):
        o = res.results[c]["out"]  # (N, S*F)
        outs.append(o.reshape(N, S, F).transpose(1, 0, 2))
    full = np.concatenate(outs, axis=0).reshape(B, T, N, F)
    return np.ascontiguousarray(full.astype(np.float32))


last_results = None
